# revision 13
# baseline (speedup 1.0000x reference)
"""Trainium2 Bass kernel for nn_Decoder (SIREN point-cloud decoder:
rotate -> SIREN -> bilinear scatter -> gaussian blur -> CTF filter).

Strategy (data-parallel over batch, 4 images per core, no collectives):
  * h = sin(30*(vec(crot)@W1+b1)) collapses algebraically: S = coords^T @
    W1.reshape(N,24) (3x24, computed once on PE), then h_pre = R:S contraction.
  * bilinear scatter: img^T accumulated in PSUM as sum over 128-point chunks of
    A^T @ D  where A[p,c]=hat(x_p-c), D[p,r]=hat(y_p-r)*delta_p,
    hat(t)=relu(1-|t|) built by DVE tensor_scalar (per-partition bias trick)
    + one ACT relu per chunk. Exactly reproduces bilinear weights + border drop.
  * gaussian blur folded into DFT matrices (host constants); rfft2/ctf/irfft2
    as 48 PE matmuls per image.
"""

import numpy as np

B = 32
N = 200000
XS = 256
F = 129
NCH = 1563           # ceil(200000/128)
NPAD = NCH * 128     # 200064
B_PER = 4            # images per core
N_CORES = 8
SCALE = 100.0
CENTER = XS / 2.0
W0 = 30.0
SIGMA = 1.0
RAD = 3

_F32 = np.float32

_CACHE = {}


def _euler_zyz(angles):
    a, b, g = angles[:, 0], angles[:, 1], angles[:, 2]

    def rz(t):
        c, s = np.cos(t), np.sin(t)
        z, o = np.zeros_like(t), np.ones_like(t)
        return np.stack([np.stack([c, -s, z], -1),
                         np.stack([s, c, z], -1),
                         np.stack([z, z, o], -1)], -2)

    def ry(t):
        c, s = np.cos(t), np.sin(t)
        z, o = np.zeros_like(t), np.ones_like(t)
        return np.stack([np.stack([c, z, s], -1),
                         np.stack([z, o, z], -1),
                         np.stack([-s, z, c], -1)], -2)

    return rz(a) @ ry(b) @ rz(g)


def _fft_consts():
    k = np.exp(-0.5 * (np.arange(-RAD, RAD + 1, dtype=np.float64) / SIGMA) ** 2)
    k = k / k.sum()
    G = np.zeros((XS, XS))
    for d in range(-RAD, RAD + 1):
        for i in range(XS):
            j = i + d
            if 0 <= j < XS:
                G[i, j] += k[d + RAD]
    ar = np.arange(XS)
    C = np.exp(-2j * np.pi * ar[:, None] * np.arange(F)[None, :] / XS)
    L0 = np.exp(-2j * np.pi * ar[:, None] * ar[None, :] / XS)
    M0 = np.exp(+2j * np.pi * ar[:, None] * ar[None, :] / XS) / XS
    s = np.full(F, 2.0); s[0] = 1.0; s[F - 1] = 1.0
    th = 2 * np.pi * ar[:, None] * np.arange(F)[None, :] / XS
    Qr = np.cos(th) * s / XS
    QiN = -np.sin(th) * s / XS
    L = L0 @ G
    Rc = G @ C

    def khalf(x):  # (256, W) -> (128, 2*W) K-chunk blocks
        w = x.shape[1]
        return np.ascontiguousarray(
            x.reshape(2, 128, w).transpose(1, 0, 2).reshape(128, 2 * w)
        ).astype(_F32)

    return {
        "rr": khalf(Rc.real), "ri": khalf(Rc.imag),
        "lt_r": khalf(L.real.T), "lt_i": khalf(L.imag.T), "lt_in": khalf(-L.imag.T),
        "mt_r": khalf(M0.real.T), "mt_i": khalf(M0.imag.T), "mt_in": khalf(-M0.imag.T),
        "qrt_m": np.ascontiguousarray(Qr.T[:128]).astype(_F32),
        "qrt_d": np.ascontiguousarray(Qr.T[128:]).astype(_F32),
        "qit_m": np.ascontiguousarray(QiN.T[:128]).astype(_F32),
        "qit_d": np.ascontiguousarray(QiN.T[128:]).astype(_F32),
    }


def _build_program(nch):
    """Trace the (input-independent) bass program. Returns finalized nc."""
    from contextlib import ExitStack
    import concourse.bass as bass
    import concourse.bacc as bacc
    import concourse.tile as tile
    from concourse import mybir

    dt = mybir.dt
    AF = mybir.ActivationFunctionType
    OP = mybir.AluOpType

    nc = bacc.Bacc("TRN2", target_bir_lowering=False)

    # ---- I/O ----
    coords_sb_d = nc.dram_tensor("coords_sb", [128, nch * 3], dt.float32, kind="ExternalInput")
    w1r_d = nc.dram_tensor("w1r", [128, nch * 24], dt.float32, kind="ExternalInput")
    w5b_d = nc.dram_tensor("w5b", [9, nch * 128], dt.float32, kind="ExternalInput")
    iota_d = nc.dram_tensor("iota", [128, 256], dt.float32, kind="ExternalInput")
    rp_d = nc.dram_tensor("rp", [128, 8 * B_PER], dt.float32, kind="ExternalInput")
    r2_d = nc.dram_tensor("r2", [3, 12], dt.float32, kind="ExternalInput")
    b1s_d = nc.dram_tensor("b1s", [8, 1], dt.float32, kind="ExternalInput")
    wh_d = nc.dram_tensor("wh", [8, 24], dt.float32, kind="ExternalInput")
    bh_d = nc.dram_tensor("bh", [8, 3], dt.float32, kind="ExternalInput")
    ctf_d = nc.dram_tensor("ctf_sb", [128, B_PER * 2 * F], dt.float32, kind="ExternalInput")
    fnames = ["rr", "ri", "lt_r", "lt_i", "lt_in", "mt_r", "mt_i", "mt_in"]
    fd = {n: nc.dram_tensor(n, [128, 512 if n.startswith(("lt", "mt")) else 258],
                            dt.float32, kind="ExternalInput") for n in fnames}
    fd["qrt_m"] = nc.dram_tensor("qrt_m", [128, 256], dt.float32, kind="ExternalInput")
    fd["qit_m"] = nc.dram_tensor("qit_m", [128, 256], dt.float32, kind="ExternalInput")
    fd["qrt_d"] = nc.dram_tensor("qrt_d", [1, 256], dt.float32, kind="ExternalInput")
    fd["qit_d"] = nc.dram_tensor("qit_d", [1, 256], dt.float32, kind="ExternalInput")
    out_d = nc.dram_tensor("out", [B_PER, 256, 256], dt.float32, kind="ExternalOutput")

    W1G = 64             # W1 chunks per DMA group
    W5G = 32             # W5 chunks per DMA group (psum evac granularity)

    with tile.TileContext(nc) as tc:
        with ExitStack() as ctx:
            const = ctx.enter_context(tc.tile_pool(name="const", bufs=1))
            stream = ctx.enter_context(tc.tile_pool(name="stream", bufs=2))
            small = ctx.enter_context(tc.tile_pool(name="small", bufs=2))
            posp = ctx.enter_context(tc.tile_pool(name="posp", bufs=2))
            scat = ctx.enter_context(tc.tile_pool(name="scat", bufs=4))
            fftp = ctx.enter_context(tc.tile_pool(name="fftp", bufs=2))
            psS = ctx.enter_context(tc.tile_pool(name="psS", bufs=1, space="PSUM"))
            psD = ctx.enter_context(tc.tile_pool(name="psD", bufs=2, space="PSUM"))
            psI = ctx.enter_context(tc.tile_pool(name="psI", bufs=1, space="PSUM"))
            psF = ctx.enter_context(tc.tile_pool(name="psF", bufs=2, space="PSUM"))
            psFd = ctx.enter_context(tc.tile_pool(name="psFd", bufs=1, space="PSUM"))

            # ---- resident constants ----
            coords = const.tile([128, nch, 3], dt.float32)
            nc.sync.dma_start(out=coords[:], in_=coords_sb_d[:].rearrange("p (c m) -> p c m", m=3))
            iota = const.tile([128, 256], dt.float32)
            nc.sync.dma_start(out=iota[:], in_=iota_d[:])
            rp = const.tile([128, 8 * B_PER], dt.float32)
            nc.sync.dma_start(out=rp[:], in_=rp_d[:])
            r2 = const.tile([3, 12], dt.float32)
            nc.sync.dma_start(out=r2[:], in_=r2_d[:])
            b1s = const.tile([8, 1], dt.float32)
            nc.sync.dma_start(out=b1s[:], in_=b1s_d[:])
            wh = const.tile([8, 24], dt.float32)
            nc.sync.dma_start(out=wh[:], in_=wh_d[:])
            bh = const.tile([8, 3], dt.float32)
            nc.sync.dma_start(out=bh[:], in_=bh_d[:])
            ctf = const.tile([128, B_PER * 2 * F], dt.float32)
            nc.sync.dma_start(out=ctf[:], in_=ctf_d[:])
            fc = {}
            for n, d in fd.items():
                t = const.tile(list(d.shape), dt.float32, tag=n)
                nc.sync.dma_start(out=t[:], in_=d[:])
                fc[n] = t

            # ---- S = coords^T @ W1r  (3x24) ----
            s_ps = psS.tile([3, 24], dt.float32, tag="s")
            ngrp = (nch + W1G - 1) // W1G
            c = 0
            for g in range(ngrp):
                gn = min(W1G, nch - g * W1G)
                w1g = stream.tile([128, W1G * 24], dt.float32, tag="w1g")
                nc.sync.dma_start(out=w1g[:, : gn * 24],
                                  in_=w1r_d[:, g * W1G * 24:(g * W1G + gn) * 24])
                for cl in range(gn):
                    nc.tensor.matmul(s_ps[:], coords[:, c, :], w1g[:, cl * 24:(cl + 1) * 24],
                                     start=(c == 0), stop=(c == nch - 1))
                    c += 1

            # ---- h chain (tiny) ----
            # hk = S-contraction with R: psum [24,12] = sum_m S[m,(k,j)] x r2[m,(b,k')]
            hk_ps = psS.tile([96, 12], dt.float32, tag="s")
            s_sb = small.tile([3, 24], dt.float32)
            nc.vector.tensor_copy(out=s_sb[:], in_=s_ps[:])
            s_sb32 = small.tile([3, 96], dt.float32)
            nc.vector.memset(s_sb32[:], 0.0)
            nc.vector.tensor_copy(
                out=s_sb32[:].rearrange("p (g j) -> p g j", j=32)[:, :, 0:8],
                in_=s_sb[:].rearrange("p (g j) -> p g j", j=8))
            nc.tensor.matmul(hk_ps[:], s_sb32[:], r2[:], start=True, stop=True)
            h_pre = small.tile([8, B_PER], dt.float32)
            nc.vector.tensor_copy(out=h_pre[:], in_=hk_ps[0:8, 0:12].rearrange("p (b k) -> p b k", k=3)[:, :, 0])
            nc.vector.tensor_tensor(out=h_pre[:], in0=h_pre[:],
                                    in1=hk_ps[32:40, 0:12].rearrange("p (b k) -> p b k", k=3)[:, :, 1],
                                    op=OP.add)
            nc.vector.tensor_tensor(out=h_pre[:], in0=h_pre[:],
                                    in1=hk_ps[64:72, 0:12].rearrange("p (b k) -> p b k", k=3)[:, :, 2],
                                    op=OP.add)
            h9 = small.tile([9, B_PER], dt.float32, tag="h9")
            nc.vector.memset(h9[:], 1.0)
            nc.scalar.activation(h9[0:8, :], h_pre[:], AF.Sin, bias=b1s[:], scale=W0)
            for i in range(3):
                hh_ps = psS.tile([8, B_PER], dt.float32, tag="s")
                nc.tensor.matmul(hh_ps[:], wh[:, 8 * i:8 * (i + 1)], h9[0:8, :], start=True, stop=True)
                nc.scalar.activation(h9[0:8, :], hh_ps[:], AF.Sin, bias=bh[:, i:i + 1], scale=1.0)

            # ---- delta = h @ W5 + b5 -> [128, (c, b)] ----
            delta = const.tile([128, nch * B_PER], dt.float32)
            ngrp5 = (nch + W5G - 1) // W5G
            for g in range(ngrp5):
                gn = min(W5G, nch - g * W5G)
                w5g = stream.tile([9, W5G * 128], dt.float32, tag="w5g")
                nc.sync.dma_start(out=w5g[:, : gn * 128],
                                  in_=w5b_d[:, g * W5G * 128:(g * W5G + gn) * 128])
                d_ps = psD.tile([128, W5G * B_PER], dt.float32, tag="dps")
                for cl in range(gn):
                    nc.tensor.matmul(d_ps[:, cl * B_PER:(cl + 1) * B_PER],
                                     w5g[:, cl * 128:(cl + 1) * 128], h9[:],
                                     start=True, stop=True)
                nc.scalar.copy(out=delta[:, g * W5G * B_PER:(g * W5G + gn) * B_PER],
                               in_=d_ps[:, : gn * B_PER])

            # ---- per-image: positions, scatter, FFT ----
            for b in range(B_PER):
                # nx/ny = -(pos) affine over coords
                nx = posp.tile([128, nch], dt.float32, tag="nx")
                ny = posp.tile([128, nch], dt.float32, tag="ny")
                for (t, o) in ((nx, 0), (ny, 4)):
                    nc.vector.tensor_scalar(out=t[:], in0=coords[:, :, 0],
                                            scalar1=rp[:, 8 * b + o:8 * b + o + 1],
                                            scalar2=rp[:, 8 * b + o + 3:8 * b + o + 4],
                                            op0=OP.mult, op1=OP.add)
                    nc.vector.scalar_tensor_tensor(out=t[:], in0=coords[:, :, 1],
                                                   scalar=rp[:, 8 * b + o + 1:8 * b + o + 2],
                                                   in1=t[:], op0=OP.mult, op1=OP.add)
                    nc.vector.scalar_tensor_tensor(out=t[:], in0=coords[:, :, 2],
                                                   scalar=rp[:, 8 * b + o + 2:8 * b + o + 3],
                                                   in1=t[:], op0=OP.mult, op1=OP.add)

                # scatter: imgT[c, r] += sum over chunks A^T @ D
                p0 = psI.tile([128, 256], dt.float32, tag="p0")
                p1 = psI.tile([128, 256], dt.float32, tag="p1")
                for cc in range(nch):
                    # u = |iota - pos|  (ACT Abs with per-partition bias)
                    u = scat.tile([128, 512], dt.float32, tag="u")
                    nc.scalar.activation(u[:, 0:256], iota[:], AF.Abs,
                                         bias=nx[:, cc:cc + 1], scale=1.0)
                    nc.scalar.activation(u[:, 256:512], iota[:], AF.Abs,
                                         bias=ny[:, cc:cc + 1], scale=1.0)
                    # a = min(u,1)-1 = -hat ; dd = (-hat_y)*delta ; signs cancel in matmul
                    ab = scat.tile([128, 512], dt.float32, tag="ab")
                    nc.vector.tensor_scalar(out=ab[:, 0:256], in0=u[:, 0:256],
                                            scalar1=1.0, scalar2=1.0,
                                            op0=OP.min, op1=OP.subtract)
                    nc.vector.tensor_scalar(out=ab[:, 256:512], in0=u[:, 256:512],
                                            scalar1=1.0, scalar2=1.0,
                                            op0=OP.min, op1=OP.subtract)
                    dd = scat.tile([128, 256], dt.float32, tag="dd")
                    nc.vector.tensor_scalar_mul(dd[:], ab[:, 256:512],
                                                delta[:, B_PER * cc + b:B_PER * cc + b + 1])
                    nc.tensor.matmul(p0[:], ab[:, 0:128], dd[:],
                                     start=(cc == 0), stop=(cc == nch - 1))
                    nc.tensor.matmul(p1[:], ab[:, 128:256], dd[:],
                                     start=(cc == 0), stop=(cc == nch - 1))

                # evac imgT: p_sb [128, (ch, r)]
                p_sb = fftp.tile([128, 512], dt.float32, tag="p_sb")
                nc.scalar.copy(out=p_sb[:, 0:256], in_=p0[:])
                nc.scalar.copy(out=p_sb[:, 256:512], in_=p1[:])

                # A: U = img @ Rc -> psum [r-half, F] x {r,i}
                usb = {}
                for comp, rc in (("r", fc["rr"]), ("i", fc["ri"])):
                    ut = fftp.tile([128, 2 * F], dt.float32, tag=f"u{comp}")
                    for rh in range(2):
                        up = psF.tile([128, F], dt.float32, tag="a")
                        for ch in range(2):
                            nc.tensor.matmul(up[:], p_sb[:, ch * 256 + rh * 128: ch * 256 + rh * 128 + 128],
                                             rc[:, ch * F:(ch + 1) * F],
                                             start=(ch == 0), stop=(ch == 1))
                        nc.scalar.copy(out=ut[:, rh * F:(rh + 1) * F], in_=up[:])
                    usb[comp] = ut

                # B: Z = L @ U ; C: T = Z * ctf  -> tsb [128, (zh, F)] x {r,i}
                tsb = {}
                for comp in ("r", "i"):
                    tt = fftp.tile([128, 2 * F], dt.float32, tag=f"t{comp}")
                    if comp == "r":
                        terms = (("lt_r", "r"), ("lt_in", "i"))
                    else:
                        terms = (("lt_r", "i"), ("lt_i", "r"))
                    for zh in range(2):
                        zp = psF.tile([128, F], dt.float32, tag="a")
                        k = 0
                        for (lname, ucomp) in terms:
                            for rh in range(2):
                                nc.tensor.matmul(zp[:], fc[lname][:, rh * 256 + zh * 128: rh * 256 + zh * 128 + 128],
                                                 usb[ucomp][:, rh * F:(rh + 1) * F],
                                                 start=(k == 0), stop=(k == 3))
                                k += 1
                        nc.vector.tensor_tensor(out=tt[:, zh * F:(zh + 1) * F], in0=zp[:],
                                                in1=ctf[:, (b * 2 + zh) * F:(b * 2 + zh + 1) * F],
                                                op=OP.mult)
                    tsb[comp] = tt

                # D: WT[f, w] = sum_z T[z, f] * Mt[z, w]   (lhsT = T slices)
                wts = {}
                for comp in ("r", "i"):
                    if comp == "r":
                        terms = (("r", "mt_r"), ("i", "mt_in"))
                    else:
                        terms = (("i", "mt_r"), ("r", "mt_i"))
                    wm = fftp.tile([128, 256], dt.float32, tag=f"wm{comp}")
                    wd = fftp.tile([1, 256], dt.float32, tag=f"wd{comp}")
                    wp = psF.tile([128, 256], dt.float32, tag="a")
                    wpd = psFd.tile([1, 256], dt.float32, tag="d")
                    k = 0
                    for (tcomp, mname) in terms:
                        for zh in range(2):
                            nc.tensor.matmul(wp[:], tsb[tcomp][:, zh * F: zh * F + 128],
                                             fc[mname][:, zh * 256:(zh + 1) * 256],
                                             start=(k == 0), stop=(k == 3))
                            k += 1
                    k = 0
                    for (tcomp, mname) in terms:
                        for zh in range(2):
                            nc.tensor.matmul(wpd[:], tsb[tcomp][:, zh * F + 128: zh * F + 129],
                                             fc[mname][:, zh * 256:(zh + 1) * 256],
                                             start=(k == 0), stop=(k == 3))
                            k += 1
                    nc.scalar.copy(out=wm[:], in_=wp[:])
                    nc.scalar.copy(out=wd[:], in_=wpd[:])
                    wts[comp] = (wm, wd)

                # E: out[n, m] = sum_f WrT[f,n] QrT[f,m] + WiT[f,n] QiT[f,m]
                osb = fftp.tile([128, 512], dt.float32, tag="osb")
                for nh in range(2):
                    op_ps = psF.tile([128, 256], dt.float32, tag="a")
                    k = 0
                    for comp, qm, qd in (("r", fc["qrt_m"], fc["qrt_d"]),
                                         ("i", fc["qit_m"], fc["qit_d"])):
                        wm, wd = wts[comp]
                        nc.tensor.matmul(op_ps[:], wm[:, nh * 128:(nh + 1) * 128], qm[:],
                                         start=(k == 0), stop=False)
                        k += 1
                        nc.tensor.matmul(op_ps[:], wd[:, nh * 128:(nh + 1) * 128], qd[:],
                                         start=False, stop=(k == 3))
                        k += 1
                    nc.scalar.copy(out=osb[:, nh * 256:(nh + 1) * 256], in_=op_ps[:])
                nc.sync.dma_start(out=out_d[b, 0:128, :], in_=osb[:, 0:256])
                nc.sync.dma_start(out=out_d[b, 128:256, :], in_=osb[:, 256:512])

    nc.finalize()
    return nc


def _get_program(nch):
    if nch not in _CACHE:
        _CACHE[nch] = _build_program(nch)
    return _CACHE[nch]


def _prep_in_maps(inp, nch):
    rows = inp["rows"]; shifts = inp["shifts"]; coords = inp["coords"]
    W1 = inp["W1"]; b1 = inp["b1"]; Wh = inp["Wh"]; bh = inp["bh"]
    W5 = inp["W5"]; b5 = inp["b5"]; ctf = inp["ctf"]
    npad = nch * 128
    npts = min(N, npad)
    R = _euler_zyz(np.asarray(rows, np.float64)).astype(np.float64)

    # ---- shared host-prepped tensors ----
    cpad = np.zeros((npad, 3), _F32)
    cpad[:npts] = np.asarray(coords, _F32)[:npts]
    coords_sb = np.ascontiguousarray(
        cpad.reshape(nch, 128, 3).transpose(1, 0, 2).reshape(128, nch * 3))
    w1pad = np.zeros((npad, 24), _F32)
    w1pad[:npts] = np.asarray(W1, _F32).reshape(N, 24)[:npts]
    w1r = np.ascontiguousarray(
        w1pad.reshape(nch, 128, 24).transpose(1, 0, 2).reshape(128, nch * 24))
    w5b = np.zeros((9, npad), _F32)
    w5b[0:8, :npts] = np.asarray(W5, _F32)[:, :npts]
    w5b[8, :npts] = np.asarray(b5, _F32)[:npts]
    iota = np.ascontiguousarray(
        np.broadcast_to(np.arange(256, dtype=_F32), (128, 256)))
    b1s = (W0 * np.asarray(b1, _F32)).reshape(8, 1).astype(_F32)
    wh = np.concatenate([np.asarray(Wh, _F32)[i] for i in range(3)], axis=1)
    bh_sb = np.ascontiguousarray(np.asarray(bh, _F32).T)
    fcn = _fft_consts()

    shared = dict(coords_sb=coords_sb, w1r=w1r, w5b=w5b, iota=iota,
                  b1s=b1s, wh=wh, bh=bh_sb, **fcn)

    in_maps = []
    for core in range(N_CORES):
        bs = slice(core * B_PER, (core + 1) * B_PER)
        Rb = R[bs]                       # (4,3,3)
        sh = np.asarray(shifts, np.float64)[bs]
        rp = np.zeros((8 * B_PER,), _F32)
        for b in range(B_PER):
            rp[8 * b + 0:8 * b + 3] = -SCALE * Rb[b, 0, :]
            rp[8 * b + 3] = sh[b, 0] - CENTER
            rp[8 * b + 4:8 * b + 7] = -SCALE * Rb[b, 1, :]
            rp[8 * b + 7] = sh[b, 1] - CENTER
        rp_t = np.ascontiguousarray(np.broadcast_to(rp, (128, 8 * B_PER)))
        # r2[m, b*3+k] = R[b, k, m]
        r2 = np.ascontiguousarray(Rb.transpose(2, 0, 1).reshape(3, 12)).astype(_F32)
        ctf_t = np.ascontiguousarray(
            np.asarray(ctf, _F32)[bs].reshape(B_PER, 2, 128, F)
            .transpose(2, 0, 1, 3).reshape(128, B_PER * 2 * F))
        m = dict(shared)
        m.update(rp=rp_t, r2=r2, ctf_sb=ctf_t)
        in_maps.append(m)

    return in_maps


def kernel(rows, shifts, coords, W1, b1, Wh, bh, W5, b5, ctf, _nch=None):
    from concourse.bass_utils import run_bass_kernel_spmd

    nch = NCH if _nch is None else _nch
    in_maps = _prep_in_maps(dict(rows=rows, shifts=shifts, coords=coords, W1=W1,
                                 b1=b1, Wh=Wh, bh=bh, W5=W5, b5=b5, ctf=ctf), nch)
    nc = _get_program(nch)
    res = run_bass_kernel_spmd(nc, in_maps, core_ids=list(range(N_CORES)))
    out = np.concatenate([res.results[c]["out"] for c in range(N_CORES)], axis=0)
    return np.ascontiguousarray(out.astype(np.float32))
